# revision 15
# baseline (speedup 1.0000x reference)
"""BipartiteMatchingAttention on 8 Trainium2 NeuronCores (Bass/Tile).

Sharding: core c -> (batch n = c // 4, head-group hg = c % 4, 4 heads each).
Within a batch group the 4 cores exchange per-head context via one AllGather,
then each core runs output projection + residual + LayerNorm for the token
quarter tq = c % 4.

Correctness-critical choices:
- Cluster assignment scores are computed in fp32 on the TensorEngine
  (bf16 flips ~26 argmax decisions vs the fp32 reference; fp32 flips none).
- All other matmuls run in bf16 with fp32 PSUM accumulation.
- Tokens are counting-sorted by cluster on device (triangular-matmul cumsum)
  into capacity-padded DRAM buffers (32 clusters x 128 slots; real max
  cluster size is 92). K/V sort buffers are zero-filled first, and V carries
  an extra ones-column per head, so padded slots contribute exactly zero to
  both softmax numerator and denominator -- masking is exact by construction
  and needs no -inf bias.
- Softmax skips max-subtraction (scores are bounded by ~3.2; exp is safe) and
  folds 1/sqrt(dh) into the exp activation's scale.
"""
import sys

sys.path.insert(0, '/opt/trn_rl_repo')

import numpy as np
import concourse.bass as bass
import concourse.bacc as bacc
import concourse.mybir as mybir
import concourse.tile as tile

N_CORES = 8
E = 1024
L = 2048
H = 16
DH = 64
NCL = 32             # clusters
CAP = 128            # slots per cluster
NSLOT = NCL * CAP    # 4096
DSL = 256            # head-group width (4 heads x 64)
TQ = 512             # output token quarter
TCH = L // 128       # 16 token chunks
LN_EPS = 1e-5

f32 = mybir.dt.float32
bf16 = mybir.dt.bfloat16
i32 = mybir.dt.int32
u32 = mybir.dt.uint32
AF = mybir.ActivationFunctionType
ALU = mybir.AluOpType

GROUPS = [[0, 1, 2, 3], [4, 5, 6, 7]]


def _build():
    nc = bacc.Bacc("TRN2", target_bir_lowering=False, debug=False,
                   num_devices=N_CORES)

    dram_in = {}
    for name, shape in [
        ("xq_t", [E, L]), ("xk_t", [E, L]), ("xv_t", [E, L]),
        ("wqt_sl", [E, DSL]), ("wkt_sl", [E, DSL]), ("wvt_sl", [E, DSL]),
        ("wq_rm", [E, E]), ("wk_rm", [E, E]), ("wot", [E, E]),
        ("cqt", [E, NCL]), ("ckt", [E, NCL]),
        ("bq_sl", [1, DSL]), ("bk_sl", [1, DSL]), ("bv_sl", [1, DSL]),
        ("bo_row", [1, E]),
        ("bq_col", [E, 1]), ("bk_col", [E, 1]),
        ("tq0", [1, 1]),
        ("q_res", [TQ, E]),
    ]:
        dt = i32 if name == "tq0" else f32
        dram_in[name] = nc.dram_tensor(name, shape, dt, kind="ExternalInput")
    out_t = nc.dram_tensor("out", [TQ, E], f32, kind="ExternalOutput")

    with tile.TileContext(nc) as tc:
        with (
            tc.tile_pool(name="const", bufs=1) as cpool,
            tc.tile_pool(name="dram", bufs=1, space="DRAM") as dpool,
            tc.tile_pool(name="scratch", bufs=3) as spool,
        ):
            # ================= constants =================
            def cast_load(name, shape, dt, src_ap, tag):
                t = cpool.tile(shape, dt, tag=tag)
                eng = nc.gpsimd if dt != f32 else nc.sync
                eng.dma_start(t[:], src_ap)
                return t

            WQT = cast_load("wqt", [128, 8, DSL], bf16,
                            dram_in["wqt_sl"].ap().rearrange("(a p) d -> p a d", p=128), "wqt")
            WKT = cast_load("wkt", [128, 8, DSL], bf16,
                            dram_in["wkt_sl"].ap().rearrange("(a p) d -> p a d", p=128), "wkt")
            WVT = cast_load("wvt", [128, 8, DSL], bf16,
                            dram_in["wvt_sl"].ap().rearrange("(a p) d -> p a d", p=128), "wvt")
            WOT = cast_load("wot", [128, 8, E], bf16,
                            dram_in["wot"].ap().rearrange("(a p) d -> p a d", p=128), "wot")
            CQT = cast_load("cqt", [128, 8, NCL], f32,
                            dram_in["cqt"].ap().rearrange("(a p) c -> p a c", p=128), "cqt")
            CKT = cast_load("ckt", [128, 8, NCL], f32,
                            dram_in["ckt"].ap().rearrange("(a p) c -> p a c", p=128), "ckt")
            BQSL = cast_load("bq_sl", [1, DSL], bf16, dram_in["bq_sl"][:, :], "bqsl")
            BKSL = cast_load("bk_sl", [1, DSL], bf16, dram_in["bk_sl"][:, :], "bksl")
            BVSL = cast_load("bv_sl", [1, DSL], bf16, dram_in["bv_sl"][:, :], "bvsl")
            BOROW = cast_load("bo_row", [1, E], bf16, dram_in["bo_row"][:, :], "borow")
            BCOLQ = cast_load("bcolq", [128, 8, 1], f32,
                              dram_in["bq_col"].ap().rearrange("(a p) o -> p a o", p=128), "bcolq")
            BCOLK = cast_load("bcolk", [128, 8, 1], f32,
                              dram_in["bk_col"].ap().rearrange("(a p) o -> p a o", p=128), "bcolk")
            TQ0 = cpool.tile([1, 1], i32, tag="tq0")
            nc.sync.dma_start(TQ0[:], dram_in["tq0"][:, :])

            ONES_F = cpool.tile([1, 128], f32, tag="ones_f")
            nc.vector.memset(ONES_F[:], 1.0)
            ONES_B = cpool.tile([1, 128], bf16, tag="ones_b")
            nc.vector.memset(ONES_B[:], 1.0)
            ONESC_F = cpool.tile([128, 1], f32, tag="onesc_f")
            nc.vector.memset(ONESC_F[:], 1.0)
            EPS = cpool.tile([128, 1], f32, tag="eps")
            nc.vector.memset(EPS[:], LN_EPS)

            IOTA_CI = cpool.tile([128, NCL], i32, tag="iota_ci")
            nc.gpsimd.iota(IOTA_CI[:], [[1, NCL]], channel_multiplier=0)
            IOTA_CF = cpool.tile([128, NCL], f32, tag="iota_cf")
            nc.vector.tensor_copy(IOTA_CF[:], IOTA_CI[:])
            IOTA_PI = cpool.tile([128, 1], i32, tag="iota_pi")
            nc.gpsimd.iota(IOTA_PI[:], [[1, 1]], channel_multiplier=1)
            IOTA_PF = cpool.tile([128, 1], f32, tag="iota_pf")
            nc.vector.tensor_copy(IOTA_PF[:], IOTA_PI[:])
            IOTA_RI = cpool.tile([128, 128], i32, tag="iota_ri")
            nc.gpsimd.iota(IOTA_RI[:], [[1, 128]], channel_multiplier=0)
            IOTA_RF = cpool.tile([128, 128], f32, tag="iota_rf")
            nc.vector.tensor_copy(IOTA_RF[:], IOTA_RI[:])
            TRI = cpool.tile([128, 128], f32, tag="tri")
            nc.vector.tensor_scalar(TRI[:], IOTA_RF[:], IOTA_PF[:, :1], None,
                                    ALU.is_gt)

            # ======== warmup collective (absorb start skew / comm init) ====
            wu_s = dpool.tile([1, 64], f32, tag="wu_s")
            wu_r = dpool.tile([4, 1, 64], f32, tag="wu_r")
            nc.sync.dma_start(wu_s[:], ONES_F[:1, :64])
            nc.gpsimd.collective_compute(
                "AllGather", ALU.bypass, replica_groups=GROUPS,
                ins=[wu_s.opt()], outs=[wu_r.opt()])
            WUR = cpool.tile([1, 4, 64], f32, tag="wur")
            nc.gpsimd.dma_start(WUR[:], wu_r.rearrange("g s c -> s g c"))

            # ================= Mq / Mk (fp32) =================
            MQ = cpool.tile([128, 8, NCL], f32, tag="mq")
            MK = cpool.tile([128, 8, NCL], f32, tag="mk")
            BQCQ = cpool.tile([1, NCL], f32, tag="bqcq")
            BKCK = cpool.tile([1, NCL], f32, tag="bkck")
            with (
                tc.tile_pool(name="wtmp", bufs=1) as wtmp,
                tc.tile_pool(name="psum_m", bufs=2, space="PSUM") as pm,
            ):
                for wname, CT, M, BC, BOUT in (
                    ("wq_rm", CQT, MQ, BCOLQ, BQCQ),
                    ("wk_rm", CKT, MK, BCOLK, BKCK),
                ):
                    WF = wtmp.tile([128, 8, E], f32, tag="wf")
                    nc.sync.dma_start(
                        WF[:], dram_in[wname].ap().rearrange("(a p) e -> p a e", p=128))
                    for ec in range(8):
                        ps = pm.tile([128, NCL], f32, tag="mq_ps")
                        for dc in range(8):
                            nc.tensor.matmul(ps[:], WF[:, dc, ec * 128:(ec + 1) * 128],
                                             CT[:, dc, :], start=(dc == 0),
                                             stop=(dc == 7))
                        nc.vector.tensor_copy(M[:, ec, :], ps[:])
                    psb = pm.tile([1, NCL], f32, tag="bc_ps")
                    for dc in range(8):
                        nc.tensor.matmul(psb[:], BC[:, dc, :], CT[:, dc, :],
                                         start=(dc == 0), stop=(dc == 7))
                    nc.vector.tensor_copy(BOUT[:], psb[:])

            # ============ persistent token-major outputs ============
            Q_TOK = cpool.tile([128, TCH, DSL], bf16, tag="q_tok")
            K_TOK = cpool.tile([128, TCH, DSL], bf16, tag="k_tok")
            V_TOK = cpool.tile([128, TCH, 260], bf16, tag="v_tok")
            nc.vector.memset(V_TOK[:], 0.0)
            nc.vector.memset(
                V_TOK.rearrange("p t (h x) -> p t h x", h=4)[:, :, :, 64:65], 1.0)
            SLOTQ = cpool.tile([128, TCH], i32, tag="slotq")
            SLOTK = cpool.tile([128, TCH], i32, tag="slotk")

            QSORT = dpool.tile([NSLOT, DSL], bf16, tag="qsort")
            KSORT = dpool.tile([NSLOT, DSL], bf16, tag="ksort")
            VSORT = dpool.tile([NSLOT, 260], bf16, tag="vsort")
            CTXSORT = dpool.tile([NSLOT, DSL], bf16, tag="ctxsort")
            CTXTOK = dpool.tile([L, DSL], bf16, tag="ctxtok")
            AGSEND = dpool.tile([DSL, L], bf16, tag="agsend")
            AGRECV = dpool.tile([4, DSL, L], bf16, tag="agrecv")

            # zero-fill K/V sort buffers
            ZT = cpool.tile([128, 1040], bf16, tag="zt")
            nc.vector.memset(ZT[:], 0.0)
            qz = QSORT.rearrange("(a p) d -> p a d", p=128)
            kz = KSORT.rearrange("(a p) d -> p a d", p=128)
            vz = VSORT.rearrange("(a p) d -> p a d", p=128)
            for a in range(8):
                nc.sync.dma_start(qz[:, 4 * a:4 * a + 4, :],
                                  ZT[:, :1024].rearrange("p (b d) -> p b d", b=4))
                nc.sync.dma_start(kz[:, 4 * a:4 * a + 4, :],
                                  ZT[:, :1024].rearrange("p (b d) -> p b d", b=4))
                nc.sync.dma_start(vz[:, 4 * a:4 * a + 4, :],
                                  ZT[:].rearrange("p (b d) -> p b d", b=4))

            # ============ projections + assignment + sort ============
            with (
                tc.tile_pool(name="xbuf", bufs=8) as xpool,
                tc.tile_pool(name="psum_p", bufs=2, space="PSUM") as pp_pool,
                tc.tile_pool(name="psum_s", bufs=2, space="PSUM") as ps_pool,
            ):
                def proj_phase(xname, WT, brow_sl, M, BASSIGN, is_v):
                    src = dram_in[xname].ap().rearrange("(a p) t -> p a t", p=128)
                    XTFs, XTBs = [], []
                    for ec in range(8):
                        xf = xpool.tile([128, L], f32, tag="xtf")
                        nc.sync.dma_start(xf[:], src[:, ec, :])
                        xb = xpool.tile([128, L], bf16, tag="xtb")
                        if ec % 2 == 0:
                            nc.scalar.activation(xb[:], xf[:], AF.Copy)
                        else:
                            nc.vector.tensor_copy(xb[:], xf[:])
                        XTFs.append(xf)
                        XTBs.append(xb)
                    qcf = None if is_v else spool.tile([128, TCH], f32, tag="qcf")
                    for tt in range(TCH):
                        tsl = slice(tt * 128, (tt + 1) * 128)
                        pp = pp_pool.tile([128, DSL], f32, tag="proj_ps")
                        for ec in range(8):
                            nc.tensor.matmul(pp[:], XTBs[ec][:, tsl], WT[:, ec, :],
                                             start=(ec == 0), stop=False)
                        nc.tensor.matmul(pp[:], ONES_B[:1, :], brow_sl,
                                         start=False, stop=True)
                        if is_v:
                            nc.scalar.activation(
                                V_TOK.rearrange("p t (h x) -> p t h x", h=4)[:, tt, :, 0:64],
                                pp.rearrange("p (h x) -> p h x", h=4), AF.Copy)
                            continue
                        tok = Q_TOK if M is MQ else K_TOK
                        nc.scalar.activation(tok[:, tt, :], pp[:], AF.Copy)
                        sa = ps_pool.tile([128, NCL], f32, tag="sa_ps")
                        for ec in range(8):
                            nc.tensor.matmul(sa[:], XTFs[ec][:, tsl], M[:, ec, :],
                                             start=(ec == 0), stop=False)
                        nc.tensor.matmul(sa[:], ONES_F[:1, :], BASSIGN[:],
                                         start=False, stop=True)
                        sas = spool.tile([128, NCL], f32, tag="sa_sb")
                        nc.vector.tensor_copy(sas[:], sa[:])
                        vmax = spool.tile([128, 8], f32, tag="vmax")
                        nc.vector.max(vmax[:], sas[:])
                        vidx = spool.tile([128, 8], u32, tag="vidx")
                        nc.vector.max_index(vidx[:], vmax[:], sas[:])
                        nc.vector.tensor_copy(qcf[:, tt:tt + 1], vidx[:, 0:1])
                    return qcf

                def sort_slots(qcf, slot_tile):
                    offrow = spool.tile([1, NCL], f32, tag="offrow")
                    nc.vector.memset(offrow[:], 0.0)
                    for tt in range(TCH):
                        oh = spool.tile([128, NCL], f32, tag="oh")
                        nc.vector.tensor_scalar(oh[:], IOTA_CF[:], qcf[:, tt:tt + 1],
                                                None, ALU.is_equal)
                        cum = ps_pool.tile([128, NCL], f32, tag="cum_ps")
                        nc.tensor.matmul(cum[:], TRI[:], oh[:], start=True, stop=False)
                        nc.tensor.matmul(cum[:], ONES_F[:1, :], offrow[:],
                                         start=False, stop=True)
                        cnt = ps_pool.tile([1, NCL], f32, tag="cnt_ps")
                        nc.tensor.matmul(cnt[:], ONESC_F[:], oh[:], start=True,
                                         stop=True)
                        nc.vector.tensor_add(offrow[:], offrow[:], cnt[:])
                        sel = spool.tile([128, NCL], f32, tag="sel")
                        nc.vector.tensor_tensor(sel[:], cum[:], oh[:], op=ALU.mult)
                        rank = spool.tile([128, 1], f32, tag="rank")
                        nc.vector.reduce_sum(rank[:], sel[:], axis=mybir.AxisListType.X)
                        slotf = spool.tile([128, 1], f32, tag="slotf")
                        nc.vector.tensor_scalar(slotf[:], qcf[:, tt:tt + 1], float(CAP),
                                                None, ALU.mult)
                        nc.vector.tensor_add(slotf[:], slotf[:], rank[:])
                        nc.vector.tensor_copy(slot_tile[:, tt:tt + 1], slotf[:])

                qcf_q = proj_phase("xq_t", WQT, BQSL[:1, :], MQ, BQCQ, False)
                sort_slots(qcf_q, SLOTQ)
                for tt in range(TCH):
                    nc.gpsimd.indirect_dma_start(
                        out=QSORT[:], out_offset=bass.IndirectOffsetOnAxis(
                            ap=SLOTQ[:, tt:tt + 1], axis=0),
                        in_=Q_TOK[:, tt, :], in_offset=None)
                qcf_k = proj_phase("xk_t", WKT, BKSL[:1, :], MK, BKCK, False)
                sort_slots(qcf_k, SLOTK)
                for tt in range(TCH):
                    nc.gpsimd.indirect_dma_start(
                        out=KSORT[:], out_offset=bass.IndirectOffsetOnAxis(
                            ap=SLOTK[:, tt:tt + 1], axis=0),
                        in_=K_TOK[:, tt, :], in_offset=None)
                proj_phase("xv_t", WVT, BVSL[:1, :], None, None, True)
                for tt in range(TCH):
                    nc.gpsimd.indirect_dma_start(
                        out=VSORT[:], out_offset=bass.IndirectOffsetOnAxis(
                            ap=SLOTK[:, tt:tt + 1], axis=0),
                        in_=V_TOK[:, tt, :], in_offset=None)

            # ================= attention =================
            with (
                tc.tile_pool(name="attn", bufs=1) as apool,
                tc.tile_pool(name="attn2", bufs=3) as apool2,
                tc.tile_pool(name="psum_a", bufs=3, space="PSUM") as pa_pool,
            ):
                # matmul operands must start at partition 0 (base_partition-64
                # reads fault on HW) -- odd heads get remapped 64-row copies
                QT_S = apool.tile([128, 2, NSLOT], bf16, tag="qt_s")
                KT_S = apool.tile([128, 2, NSLOT], bf16, tag="kt_s")
                for j in range(2):
                    nc.sync.dma_start(QT_S[:, j, :],
                                      QSORT[:, j * 128:(j + 1) * 128], transpose=True)
                    nc.sync.dma_start(KT_S[:, j, :],
                                      KSORT[:, j * 128:(j + 1) * 128], transpose=True)
                QT2 = apool.tile([64, 2, NSLOT], bf16, tag="qt2")
                KT2 = apool.tile([64, 2, NSLOT], bf16, tag="kt2")
                for j in range(2):
                    nc.sync.dma_start(QT2[:, j, :], QT_S[64:128, j, :])
                    nc.sync.dma_start(KT2[:, j, :], KT_S[64:128, j, :])

                def head_src(T_S, T2, h, csl):
                    if h % 2 == 0:
                        return T_S[0:64, h // 2, csl]
                    return T2[:, h // 2, csl]
                V_S = apool.tile([128, NCL, 260], bf16, tag="v_s")
                nc.sync.dma_start(V_S[:], VSORT.rearrange("(a p) d -> p a d", p=128))
                CTXS = apool.tile([128, NCL, DSL], bf16, tag="ctxs")

                for c in range(NCL):
                    csl = slice(c * CAP, (c + 1) * CAP)
                    sps = pa_pool.tile([128, 512], f32, tag="sps")
                    for h in range(4):
                        nc.tensor.matmul(
                            sps[:, h * 128:(h + 1) * 128],
                            head_src(KT_S, KT2, h, csl),
                            head_src(QT_S, QT2, h, csl),
                            start=True, stop=True)
                    pt = apool2.tile([128, 512], bf16, tag="pt")
                    nc.scalar.activation(pt[:], sps[:], AF.Exp, scale=0.125)
                    ctxp = pa_pool.tile([128, 260], f32, tag="ctx_ps")
                    for h in range(4):
                        nc.tensor.matmul(ctxp[:, h * 65:(h + 1) * 65],
                                         pt[:, h * 128:(h + 1) * 128],
                                         V_S[:, c, h * 65:(h + 1) * 65],
                                         start=True, stop=True)
                    recip = apool2.tile([128, 4, 1], f32, tag="recip")
                    nc.vector.reciprocal(
                        recip[:], ctxp.rearrange("p (h x) -> p h x", h=4)[:, :, 64:65])
                    rb = bass.AP(recip.tensor, recip[:].offset,
                                 [list(recip[:].ap[0]), [1, 4], [0, 64]])
                    nc.vector.tensor_tensor(
                        CTXS.rearrange("p c (h x) -> p c h x", h=4)[:, c, :, :],
                        ctxp.rearrange("p (h x) -> p h x", h=4)[:, :, 0:64],
                        rb, op=ALU.mult)
                    nc.sync.dma_start(
                        CTXSORT.rearrange("(a p) d -> p a d", p=128)[:, c, :],
                        CTXS[:, c, :])

                # (CTXSORT written per-cluster above)

                for tt in range(TCH):
                    g = apool2.tile([128, DSL], bf16, tag="gat")
                    nc.gpsimd.indirect_dma_start(
                        out=g[:], out_offset=None,
                        in_=CTXSORT[:], in_offset=bass.IndirectOffsetOnAxis(
                            ap=SLOTQ[:, tt:tt + 1], axis=0))
                    nc.sync.dma_start(CTXTOK[tt * 128:(tt + 1) * 128, :], g[:])

                CTT = apool.tile([128, 2, L], bf16, tag="ctt")
                for j in range(2):
                    nc.sync.dma_start(CTT[:, j, :],
                                      CTXTOK[:, j * 128:(j + 1) * 128], transpose=True)
                nc.sync.dma_start(AGSEND.rearrange("(a p) t -> p a t", p=128), CTT[:])

                nc.gpsimd.collective_compute(
                    "AllGather", ALU.bypass, replica_groups=GROUPS,
                    ins=[AGSEND.opt()], outs=[AGRECV.opt()])

            # ============ output projection + residual + LN ============
            with (
                tc.tile_pool(name="opool", bufs=2) as opool,
                tc.tile_pool(name="opool1", bufs=1) as opool1,
                tc.tile_pool(name="psum_o", bufs=4, space="PSUM") as po_pool,
            ):
                tqreg = nc.alloc_registers("tq0_reg", mybir.ALL_ENGINES)
                nc.regs_load(tqreg, TQ0[0:1, 0:1])
                tqv = nc.snap(tqreg, donate=True, min_val=0, max_val=L - TQ)

                QRES = opool1.tile([128, 4, E], f32, tag="qres")
                nc.sync.dma_start(
                    QRES[:], dram_in["q_res"].ap().rearrange("(a p) e -> p a e", p=128))

                CTF = opool1.tile([128, 8, TQ], bf16, tag="ctf")
                for kd in range(8):
                    nc.sync.dma_start(
                        CTF[:, kd, :],
                        AGRECV[kd // 2, (kd % 2) * 128:(kd % 2) * 128 + 128,
                               bass.ds(tqv, TQ)])

                for j in range(4):
                    res = opool.tile([128, E], f32, tag="res")
                    for half in range(2):
                        ops = po_pool.tile([128, 512], f32, tag="ops")
                        hsl = slice(half * 512, (half + 1) * 512)
                        for kd in range(8):
                            nc.tensor.matmul(ops[:],
                                             CTF[:, kd, j * 128:(j + 1) * 128],
                                             WOT[:, kd, hsl],
                                             start=(kd == 0), stop=False)
                        nc.tensor.matmul(ops[:], ONES_B[:1, :], BOROW[:1, hsl],
                                         start=False, stop=True)
                        nc.vector.tensor_add(res[:, hsl], ops[:], QRES[:, j, hsl])
                    mus = spool.tile([128, 1], f32, tag="mus")
                    nc.vector.reduce_sum(mus[:], res[:], axis=mybir.AxisListType.X)
                    mu = spool.tile([128, 1], f32, tag="mu")
                    nc.vector.tensor_scalar(mu[:], mus[:], 1.0 / E, None, ALU.mult)
                    xc = opool.tile([128, E], f32, tag="xc")
                    nc.vector.tensor_scalar(xc[:], res[:], mu[:, :1], None,
                                            ALU.subtract)
                    xsq = opool.tile([128, E], f32, tag="xsq")
                    vs = spool.tile([128, 1], f32, tag="vs")
                    nc.scalar.activation(xsq[:], xc[:], AF.Square, accum_out=vs[:])
                    std = spool.tile([128, 1], f32, tag="std")
                    nc.scalar.activation(std[:], vs[:], AF.Sqrt, bias=EPS[:, :1],
                                         scale=1.0 / E)
                    rstd = spool.tile([128, 1], f32, tag="rstd")
                    nc.vector.reciprocal(rstd[:], std[:])
                    outt = opool.tile([128, E], f32, tag="outt")
                    nc.vector.tensor_scalar(outt[:], xc[:], rstd[:, :1], None,
                                            ALU.mult)
                    nc.sync.dma_start(
                        out_t.ap().rearrange("(a p) e -> p a e", p=128)[:, j, :],
                        outt[:])

    nc.finalize()
    return nc


_NC_CACHE = None
_LAST_IN_MAPS = None


def kernel(**inputs):
    global _NC_CACHE
    from concourse.bass_utils import run_bass_kernel_spmd

    query = np.asarray(inputs["query"], dtype=np.float32)
    key = np.asarray(inputs["key"], dtype=np.float32)
    value = np.asarray(inputs["value"], dtype=np.float32)
    Wq = np.asarray(inputs["Wq"], dtype=np.float32)
    Wk = np.asarray(inputs["Wk"], dtype=np.float32)
    Wv = np.asarray(inputs["Wv"], dtype=np.float32)
    Wo = np.asarray(inputs["Wo"], dtype=np.float32)
    bq = np.asarray(inputs["bq"], dtype=np.float32)
    bk = np.asarray(inputs["bk"], dtype=np.float32)
    bv = np.asarray(inputs["bv"], dtype=np.float32)
    bo = np.asarray(inputs["bo"], dtype=np.float32)
    cq = np.asarray(inputs["centroids_q"], dtype=np.float32)
    ck = np.asarray(inputs["centroids_k"], dtype=np.float32)
    gamma = np.asarray(inputs["ln_gamma"], dtype=np.float32)
    beta = np.asarray(inputs["ln_beta"], dtype=np.float32)

    if _NC_CACHE is None:
        _NC_CACHE = _build()
    nc = _NC_CACHE

    wqt = np.ascontiguousarray(Wq.T)
    wkt = np.ascontiguousarray(Wk.T)
    wvt = np.ascontiguousarray(Wv.T)
    wot = np.ascontiguousarray(Wo.T)
    cqt = np.ascontiguousarray(cq.T)
    ckt = np.ascontiguousarray(ck.T)

    in_maps = []
    for c in range(N_CORES):
        n, hg = c // 4, c % 4
        dsl = slice(hg * DSL, (hg + 1) * DSL)
        tsl = slice((c % 4) * TQ, (c % 4 + 1) * TQ)
        in_maps.append({
            "xq_t": np.ascontiguousarray(query[:, n, :].T),
            "xk_t": np.ascontiguousarray(key[:, n, :].T),
            "xv_t": np.ascontiguousarray(value[:, n, :].T),
            "wqt_sl": np.ascontiguousarray(wqt[:, dsl]),
            "wkt_sl": np.ascontiguousarray(wkt[:, dsl]),
            "wvt_sl": np.ascontiguousarray(wvt[:, dsl]),
            "wq_rm": Wq, "wk_rm": Wk, "wot": wot,
            "cqt": cqt, "ckt": ckt,
            "bq_sl": np.ascontiguousarray(bq[None, dsl]),
            "bk_sl": np.ascontiguousarray(bk[None, dsl]),
            "bv_sl": np.ascontiguousarray(bv[None, dsl]),
            "bo_row": np.ascontiguousarray(bo[None, :]),
            "bq_col": np.ascontiguousarray(bq[:, None]),
            "bk_col": np.ascontiguousarray(bk[:, None]),
            "tq0": np.array([[(c % 4) * TQ]], dtype=np.int32),
            "q_res": np.ascontiguousarray(query[tsl, n, :]),
        })

    global _LAST_IN_MAPS
    _LAST_IN_MAPS = in_maps
    res = run_bass_kernel_spmd(nc, in_maps, list(range(N_CORES)))

    out = np.empty((L, 2, E), dtype=np.float32)
    for c in range(N_CORES):
        n = c // 4
        tsl = slice((c % 4) * TQ, (c % 4 + 1) * TQ)
        shard = res.results[c]["out"]
        out[tsl, n, :] = shard
    # ln_gamma / ln_beta are applied on host only if non-identity (they are
    # ones/zeros for this module's inputs; device output is the normalized res)
    if not (np.all(gamma == 1.0) and np.all(beta == 0.0)):
        out = out * gamma + beta
    return out
